# revision 11
# baseline (speedup 1.0000x reference)
"""Trainium2 Bass kernel for nn_ExpandingLinear.

Reference computation:
    x_exp = concat([x, x[:, p0] * v0, x_exp1[:, p1] * v1], axis=1)   # [B, 2176]
    W     = scatter_add(weight_vals at [weight_rows, weight_cols])    # [2048, 2176]
    b     = scatter_add(bias_vals at bias_idx)                        # [2048]
    out   = x_exp @ W.T + b                                           # [B, 2048]

Every expanded feature c is a_c * x[:, q_c] for a resolvable (q_c, a_c)
(parent chains only reference earlier features), so the embed columns fold
into the base weight on the host:
    W_eff[o, q_c] += a_c * W[o, 2048 + c]      ->  out = x @ W_eff.T + b
which reduces the device work to a dense [1024, 2048] @ [2048, 2048]
matmul + bias per core (data-parallel batch shard, 8 cores).

Numerics: x and W_eff are cast to bf16 on the host (PSUM accumulates fp32);
measured end-to-end rel err ~4e-3 against the fp32 reference, well inside
the 2e-2 gate, and bf16 halves every DMA stream vs fp32.

Device schedule (per core):
  - wt (W_eff^T, [16 k-tiles, 128, 2048]) and xt (x^T, [16, 128, 1024])
    stream into resident SBUF tiles as 256KB k-tiles on 4 queues:
    W n-cols 0:1024 on sync/scalar (k even/odd), x on gpsimd/vector.
    W n-cols 1024:2048 and the bias queue behind those (needed later).
  - round n=0 runs k-outer / m-inner, paced by the k-ordered W/x streams;
    all 8 PSUM banks accumulate one m-tile each.
  - rounds n=1..3 run m-outer / k-inner from resident SBUF, staggering
    PSUM completion so evac (vector add bias -> bf16) + store overlap
    the next m-block's matmuls.
"""

import numpy as np
from contextlib import ExitStack

OUT = 2048
IN_BASE = 2048
N_EMBED = 64
IN_TOT = IN_BASE + 2 * N_EMBED  # 2176
BATCH = 8192
N_CORES = 8
B_CORE = BATCH // N_CORES       # 1024
P = 128
K_TILES = IN_BASE // P          # 16 (embeds folded away)
M_TILES = B_CORE // P           # 8
N_SPLIT = 4                     # 2048 out cols in 4 x 512 (one PSUM bank each)
NW = 512

_CACHED = {}


def _build_nc():
    import concourse.mybir as mybir
    import concourse.tile as tile
    from concourse import bacc

    f32 = mybir.dt.float32
    bf16 = mybir.dt.bfloat16

    nc = bacc.Bacc("TRN2", target_bir_lowering=False, debug=False,
                   num_devices=N_CORES)

    xt = nc.dram_tensor("xt", [IN_BASE, B_CORE], bf16, kind="ExternalInput")
    wt = nc.dram_tensor("wt", [IN_BASE, OUT], bf16, kind="ExternalInput")
    bias = nc.dram_tensor("bias", [1, OUT], f32, kind="ExternalInput")
    out = nc.dram_tensor("out", [B_CORE, OUT], bf16, kind="ExternalOutput")

    xt_ap = xt.ap().rearrange("(k p) b -> p k b", p=P)   # [128, 16, 1024]
    wt_ap = wt.ap().rearrange("(k p) n -> p k n", p=P)   # [128, 16, 2048]

    NHALF = OUT // 2  # 1024

    with tile.TileContext(nc) as tc:
        with ExitStack() as ctx:
            big_pool = ctx.enter_context(tc.tile_pool(name="big", bufs=1))
            out_pool = ctx.enter_context(tc.tile_pool(name="out", bufs=4))
            psum_pool = ctx.enter_context(
                tc.tile_pool(name="psum", bufs=8, space="PSUM"))

            wt_sb = big_pool.tile([P, K_TILES * OUT], bf16, tag="wt")
            xt_sb = big_pool.tile([P, K_TILES * B_CORE], bf16, tag="xt")
            bias_row = big_pool.tile([1, OUT], f32, tag="bias_row")
            bias_t = big_pool.tile([P, OUT], f32, tag="bias")

            # PE p-state warmup: the Tensor engine ramps 0.65 -> 1.2 ->
            # 2.4 GHz over ~3us of continuous execution. Small junk
            # matmuls issued before the input streams land burn the ramp
            # so the real matmuls start at full clock. 256-row moving dim
            # keeps the preemption granularity ~0.1-0.2us.
            warm_pool = ctx.enter_context(tc.tile_pool(name="warm", bufs=1))
            wx = warm_pool.tile([P, P], bf16, tag="wx")
            ww = warm_pool.tile([P, 256], bf16, tag="ww")
            nc.vector.memset(wx[:], 0.0)
            nc.vector.memset(ww[:], 0.0)
            wps = psum_pool.tile([P, NW], f32, tag="ps", name="warm_ps")
            for _ in range(36):
                nc.tensor.matmul(wps[:, 0:256], lhsT=wx[:], rhs=ww[:],
                                 start=True, stop=True)

            # k-ordered streams; first tiles of each are the PE-start
            # critical path: x k0 alone at the head of sync, W k0 n-cols
            # 0:512 alone at the head of scalar. Round 0 consumes x
            # k-tiles (256KB per 1.7us k-step) split over sync+gpsimd and
            # the W n0 quarter (128KB/step) on scalar. Queue FIFO order
            # doubles as priority: the n1 quarter and n-half-B chunks
            # drain only after the streams they queue behind.
            nc.sync.dma_start(
                out=xt_sb[:, 0:B_CORE], in_=xt_ap[:, 0, :])
            nc.scalar.dma_start(
                out=wt_sb[:, 0:NW], in_=wt_ap[:, 0, 0:NW])
            nc.sync.dma_start(out=bias_row[:], in_=bias.ap())
            for k in range(1, K_TILES):
                eng = nc.gpsimd if k % 2 == 0 else nc.sync
                eng.dma_start(
                    out=xt_sb[:, k * B_CORE:(k + 1) * B_CORE],
                    in_=xt_ap[:, k, :])
            for k in range(1, K_TILES):
                nc.scalar.dma_start(
                    out=wt_sb[:, k * OUT:k * OUT + NW],
                    in_=wt_ap[:, k, 0:NW])
            for k in range(K_TILES):
                nc.scalar.dma_start(
                    out=wt_sb[:, k * OUT + NW:k * OUT + NHALF],
                    in_=wt_ap[:, k, NW:NHALF])
            for k in range(K_TILES):
                eng = nc.scalar if k % 2 == 0 else nc.sync
                eng.dma_start(
                    out=wt_sb[:, k * OUT + NHALF:(k + 1) * OUT],
                    in_=wt_ap[:, k, NHALF:OUT])
            nc.gpsimd.partition_broadcast(bias_t[:], bias_row[:])

            def lhsT(k, m):
                return xt_sb[:, k * B_CORE + m * P:k * B_CORE + (m + 1) * P]

            def rhs(k, n):
                return wt_sb[:, k * OUT + n * NW:k * OUT + (n + 1) * NW]

            def evac(psum, m, n):
                ot = out_pool.tile([P, NW], bf16, tag="ot")
                nc.vector.tensor_add(
                    ot[:], psum[:], bias_t[:, n * NW:(n + 1) * NW])
                # late-round stores ride the HWDGE queues, idle once the
                # W stream ends — the SWDGE (gpsimd) end-of-kernel drain
                # is ~5us when stores are still in its queue
                if n >= 2:
                    st = nc.sync if m % 2 == 0 else nc.scalar
                else:
                    st = nc.gpsimd
                st.dma_start(
                    out=out.ap()[m * P:(m + 1) * P, n * NW:(n + 1) * NW],
                    in_=ot[:])

            # round n=0: k-outer so PE consumes W/x k-tiles in arrival
            # order (~240 GB/s joint stream pace, under the ~350 measured)
            psums = [psum_pool.tile([P, NW], f32, tag="ps",
                                    name=f"ps_n0_m{m}")
                     for m in range(M_TILES)]
            for k in range(K_TILES):
                for m in range(M_TILES):
                    nc.tensor.matmul(
                        psums[m][:], lhsT=lhsT(k, m), rhs=rhs(k, 0),
                        start=(k == 0), stop=(k == K_TILES - 1))
            for m in range(M_TILES):
                evac(psums[m], m, 0)

            # rounds n=1..3: resident SBUF, m-outer staggers psum
            # completion so evac+store overlap the next m-block
            for n in range(1, N_SPLIT):
                for m in range(M_TILES):
                    ps = psum_pool.tile([P, NW], f32, tag="ps",
                                        name=f"ps_n{n}_m{m}")
                    for k in range(K_TILES):
                        nc.tensor.matmul(
                            ps[:], lhsT=lhsT(k, m), rhs=rhs(k, n),
                            start=(k == 0), stop=(k == K_TILES - 1))
                    evac(ps, m, n)

    nc.compile()
    return nc


def _host_prep(inputs):
    import ml_dtypes

    x = np.asarray(inputs["x"], dtype=np.float32)
    wv = np.asarray(inputs["weight_vals"], dtype=np.float32)
    wr = np.asarray(inputs["weight_rows"]).astype(np.int64)
    wc = np.asarray(inputs["weight_cols"]).astype(np.int64)
    bv = np.asarray(inputs["bias_vals"], dtype=np.float32)
    bi = np.asarray(inputs["bias_idx"]).astype(np.int64)
    e0v = np.asarray(inputs["embed0_vals"], dtype=np.float32)
    e0p = np.asarray(inputs["embed0_parents"]).astype(np.int64)
    e1v = np.asarray(inputs["embed1_vals"], dtype=np.float32)
    e1p = np.asarray(inputs["embed1_parents"]).astype(np.int64)

    # dense W^T [IN_TOT, OUT] (coalesce: duplicates sum)
    wt_full = np.bincount(wc * OUT + wr, weights=wv,
                          minlength=IN_TOT * OUT).reshape(IN_TOT, OUT)

    # resolve embed parent chains to (row-in-x, multiplier), then fold the
    # expanded-feature rows of W^T into their parent rows
    q = np.empty(2 * N_EMBED, dtype=np.int64)
    a = np.empty(2 * N_EMBED, dtype=np.float64)
    q[:N_EMBED] = e0p
    a[:N_EMBED] = e0v
    for j in range(N_EMBED):
        p = int(e1p[j])
        if p < IN_BASE:
            q[N_EMBED + j] = p
            a[N_EMBED + j] = e1v[j]
        else:
            t = p - IN_BASE
            q[N_EMBED + j] = e0p[t]
            a[N_EMBED + j] = e1v[j] * e0v[t]
    wt_eff = wt_full[:IN_BASE]
    np.add.at(wt_eff, q, a[:, None] * wt_full[IN_BASE:])
    wt_bf = np.ascontiguousarray(wt_eff.astype(ml_dtypes.bfloat16))

    b = np.bincount(bi, weights=bv, minlength=OUT).astype(np.float32)
    bias_row = np.ascontiguousarray(b[None, :])

    x_bf = x.astype(ml_dtypes.bfloat16)
    xts = [np.ascontiguousarray(x_bf[i * B_CORE:(i + 1) * B_CORE].T)
           for i in range(N_CORES)]
    return xts, wt_bf, bias_row


def kernel(**inputs) -> np.ndarray:
    import time
    from concourse.bass_utils import run_bass_kernel_spmd

    if "nc" not in _CACHED:
        _CACHED["nc"] = _build_nc()
    nc = _CACHED["nc"]

    xts, wt_bf, bias_row = _host_prep(inputs)
    in_maps = [dict(xt=xts[i], wt=wt_bf, bias=bias_row)
               for i in range(N_CORES)]
    res = None
    last_exc = None
    for attempt in range(3):
        try:
            res = run_bass_kernel_spmd(nc, in_maps,
                                       core_ids=list(range(N_CORES)))
            break
        except Exception as e:  # transient device/runtime hiccups
            last_exc = e
            time.sleep(2.0)
    if res is None:
        raise last_exc
    out = np.concatenate([res.results[i]["out"] for i in range(N_CORES)],
                         axis=0)
    return np.ascontiguousarray(out.astype(np.float32))


# revision 12
# speedup vs baseline: 1.0640x; 1.0640x over previous
"""Trainium2 Bass kernel for nn_ExpandingLinear.

Reference computation:
    x_exp = concat([x, x[:, p0] * v0, x_exp1[:, p1] * v1], axis=1)   # [B, 2176]
    W     = scatter_add(weight_vals at [weight_rows, weight_cols])    # [2048, 2176]
    b     = scatter_add(bias_vals at bias_idx)                        # [2048]
    out   = x_exp @ W.T + b                                           # [B, 2048]

Every expanded feature c is a_c * x[:, q_c] for a resolvable (q_c, a_c)
(parent chains only reference earlier features), so the embed columns fold
into the base weight on the host:
    W_eff[o, q_c] += a_c * W[o, 2048 + c]      ->  out = x @ W_eff.T + b
which reduces the device work to a dense [1024, 2048] @ [2048, 2048]
matmul + bias per core (data-parallel batch shard, 8 cores).

Numerics: x and W_eff are cast to bf16 on the host (PSUM accumulates fp32);
measured end-to-end rel err ~4e-3 against the fp32 reference, well inside
the 2e-2 gate, and bf16 halves every DMA stream vs fp32.

Device schedule (per core):
  - wt (W_eff^T, [16 k-tiles, 128, 2048]) and xt (x^T, [16, 128, 1024])
    stream into resident SBUF tiles as 256KB k-tiles on 4 queues:
    W n-cols 0:1024 on sync/scalar (k even/odd), x on gpsimd/vector.
    W n-cols 1024:2048 and the bias queue behind those (needed later).
  - round n=0 runs k-outer / m-inner, paced by the k-ordered W/x streams;
    all 8 PSUM banks accumulate one m-tile each.
  - rounds n=1..3 run m-outer / k-inner from resident SBUF, staggering
    PSUM completion so evac (vector add bias -> bf16) + store overlap
    the next m-block's matmuls.
"""

import numpy as np
from contextlib import ExitStack

OUT = 2048
IN_BASE = 2048
N_EMBED = 64
IN_TOT = IN_BASE + 2 * N_EMBED  # 2176
BATCH = 8192
N_CORES = 8
B_CORE = BATCH // N_CORES       # 1024
P = 128
K_TILES = IN_BASE // P          # 16 (embeds folded away)
M_TILES = B_CORE // P           # 8
N_SPLIT = 4                     # 2048 out cols in 4 x 512 (one PSUM bank each)
NW = 512

_CACHED = {}


def _build_nc():
    import concourse.mybir as mybir
    import concourse.tile as tile
    from concourse import bacc

    f32 = mybir.dt.float32
    bf16 = mybir.dt.bfloat16

    nc = bacc.Bacc("TRN2", target_bir_lowering=False, debug=False,
                   num_devices=N_CORES)

    xt = nc.dram_tensor("xt", [IN_BASE, B_CORE], bf16, kind="ExternalInput")
    wt = nc.dram_tensor("wt", [IN_BASE, OUT], bf16, kind="ExternalInput")
    bias = nc.dram_tensor("bias", [1, OUT], f32, kind="ExternalInput")
    out = nc.dram_tensor("out", [B_CORE, OUT], bf16, kind="ExternalOutput")

    xt_ap = xt.ap().rearrange("(k p) b -> p k b", p=P)   # [128, 16, 1024]
    wt_ap = wt.ap().rearrange("(k p) n -> p k n", p=P)   # [128, 16, 2048]

    NHALF = OUT // 2  # 1024

    with tile.TileContext(nc) as tc:
        with ExitStack() as ctx:
            big_pool = ctx.enter_context(tc.tile_pool(name="big", bufs=1))
            out_pool = ctx.enter_context(tc.tile_pool(name="out", bufs=4))
            psum_pool = ctx.enter_context(
                tc.tile_pool(name="psum", bufs=8, space="PSUM"))

            wt_sb = big_pool.tile([P, K_TILES * OUT], bf16, tag="wt")
            xt_sb = big_pool.tile([P, K_TILES * B_CORE], bf16, tag="xt")
            bias_row = big_pool.tile([1, OUT], f32, tag="bias_row")
            bias_t = big_pool.tile([P, OUT], f32, tag="bias")

            # PE p-state warmup: the Tensor engine ramps 0.65 -> 1.2 ->
            # 2.4 GHz over ~3us of continuous execution. Small junk
            # matmuls issued before the input streams land burn the ramp
            # so the real matmuls start at full clock. 256-row moving dim
            # keeps the preemption granularity ~0.1-0.2us.
            warm_pool = ctx.enter_context(tc.tile_pool(name="warm", bufs=1))
            wx = warm_pool.tile([P, P], bf16, tag="wx")
            ww = warm_pool.tile([P, 256], bf16, tag="ww")
            nc.vector.memset(wx[:], 0.0)
            nc.vector.memset(ww[:], 0.0)
            wps = psum_pool.tile([P, NW], f32, tag="ps", name="warm_ps")
            for _ in range(20):
                nc.tensor.matmul(wps[:, 0:256], lhsT=wx[:], rhs=ww[:],
                                 start=True, stop=True)

            # k-ordered streams paced for round 0: x k-tiles (256KB per
            # ~1.7us k-step) on gpsimd, the W n0 quarter (128KB/step) on
            # scalar; n1 and the n-half-B chunks queue behind on sync/
            # scalar for rounds 1-3. dma_start CREATION order tracks
            # consumption order: the runtime round-robins a small pool of
            # DMA-completion semaphores over dma_starts in program order,
            # so a transfer can only begin once the 8-ago transfer is
            # done — out-of-need-order issues serialize the critical path
            # behind bulk prefetch.
            nc.sync.dma_start(
                out=xt_sb[:, 0:B_CORE], in_=xt_ap[:, 0, :])
            nc.scalar.dma_start(
                out=wt_sb[:, 0:NW], in_=wt_ap[:, 0, 0:NW])
            nc.sync.dma_start(out=bias_row[:], in_=bias.ap())
            for k in range(1, K_TILES):
                nc.gpsimd.dma_start(
                    out=xt_sb[:, k * B_CORE:(k + 1) * B_CORE],
                    in_=xt_ap[:, k, :])
                nc.scalar.dma_start(
                    out=wt_sb[:, k * OUT:k * OUT + NW],
                    in_=wt_ap[:, k, 0:NW])
            for k in range(K_TILES):
                nc.sync.dma_start(
                    out=wt_sb[:, k * OUT + NW:k * OUT + NHALF],
                    in_=wt_ap[:, k, NW:NHALF])
            for k in range(K_TILES):
                eng = nc.scalar if k % 2 == 0 else nc.sync
                eng.dma_start(
                    out=wt_sb[:, k * OUT + NHALF:(k + 1) * OUT],
                    in_=wt_ap[:, k, NHALF:OUT])
            nc.gpsimd.partition_broadcast(bias_t[:], bias_row[:])

            def lhsT(k, m):
                return xt_sb[:, k * B_CORE + m * P:k * B_CORE + (m + 1) * P]

            def rhs(k, n):
                return wt_sb[:, k * OUT + n * NW:k * OUT + (n + 1) * NW]

            def evac(psum, m, n):
                ot = out_pool.tile([P, NW], bf16, tag="ot")
                nc.vector.tensor_add(
                    ot[:], psum[:], bias_t[:, n * NW:(n + 1) * NW])
                # late-round stores ride the HWDGE queues, idle once the
                # W stream ends — the SWDGE (gpsimd) end-of-kernel drain
                # is ~5us when stores are still in its queue
                if n >= 2:
                    st = nc.sync if m % 2 == 0 else nc.scalar
                else:
                    st = nc.gpsimd
                st.dma_start(
                    out=out.ap()[m * P:(m + 1) * P, n * NW:(n + 1) * NW],
                    in_=ot[:])

            # round n=0: k-outer so PE consumes W/x k-tiles in arrival
            # order (~240 GB/s joint stream pace, under the ~350 measured)
            psums = [psum_pool.tile([P, NW], f32, tag="ps",
                                    name=f"ps_n0_m{m}")
                     for m in range(M_TILES)]
            for k in range(K_TILES):
                for m in range(M_TILES):
                    nc.tensor.matmul(
                        psums[m][:], lhsT=lhsT(k, m), rhs=rhs(k, 0),
                        start=(k == 0), stop=(k == K_TILES - 1))
            for m in range(M_TILES):
                evac(psums[m], m, 0)

            # rounds n=1..3: resident SBUF, m-outer staggers psum
            # completion so evac+store overlap the next m-block
            for n in range(1, N_SPLIT):
                for m in range(M_TILES):
                    ps = psum_pool.tile([P, NW], f32, tag="ps",
                                        name=f"ps_n{n}_m{m}")
                    for k in range(K_TILES):
                        nc.tensor.matmul(
                            ps[:], lhsT=lhsT(k, m), rhs=rhs(k, n),
                            start=(k == 0), stop=(k == K_TILES - 1))
                    evac(ps, m, n)

    nc.compile()
    return nc


def _host_prep(inputs):
    import ml_dtypes

    x = np.asarray(inputs["x"], dtype=np.float32)
    wv = np.asarray(inputs["weight_vals"], dtype=np.float32)
    wr = np.asarray(inputs["weight_rows"]).astype(np.int64)
    wc = np.asarray(inputs["weight_cols"]).astype(np.int64)
    bv = np.asarray(inputs["bias_vals"], dtype=np.float32)
    bi = np.asarray(inputs["bias_idx"]).astype(np.int64)
    e0v = np.asarray(inputs["embed0_vals"], dtype=np.float32)
    e0p = np.asarray(inputs["embed0_parents"]).astype(np.int64)
    e1v = np.asarray(inputs["embed1_vals"], dtype=np.float32)
    e1p = np.asarray(inputs["embed1_parents"]).astype(np.int64)

    # dense W^T [IN_TOT, OUT] (coalesce: duplicates sum)
    wt_full = np.bincount(wc * OUT + wr, weights=wv,
                          minlength=IN_TOT * OUT).reshape(IN_TOT, OUT)

    # resolve embed parent chains to (row-in-x, multiplier), then fold the
    # expanded-feature rows of W^T into their parent rows
    q = np.empty(2 * N_EMBED, dtype=np.int64)
    a = np.empty(2 * N_EMBED, dtype=np.float64)
    q[:N_EMBED] = e0p
    a[:N_EMBED] = e0v
    for j in range(N_EMBED):
        p = int(e1p[j])
        if p < IN_BASE:
            q[N_EMBED + j] = p
            a[N_EMBED + j] = e1v[j]
        else:
            t = p - IN_BASE
            q[N_EMBED + j] = e0p[t]
            a[N_EMBED + j] = e1v[j] * e0v[t]
    wt_eff = wt_full[:IN_BASE]
    np.add.at(wt_eff, q, a[:, None] * wt_full[IN_BASE:])
    wt_bf = np.ascontiguousarray(wt_eff.astype(ml_dtypes.bfloat16))

    b = np.bincount(bi, weights=bv, minlength=OUT).astype(np.float32)
    bias_row = np.ascontiguousarray(b[None, :])

    x_bf = x.astype(ml_dtypes.bfloat16)
    xts = [np.ascontiguousarray(x_bf[i * B_CORE:(i + 1) * B_CORE].T)
           for i in range(N_CORES)]
    return xts, wt_bf, bias_row


def kernel(**inputs) -> np.ndarray:
    import time
    from concourse.bass_utils import run_bass_kernel_spmd

    if "nc" not in _CACHED:
        _CACHED["nc"] = _build_nc()
    nc = _CACHED["nc"]

    xts, wt_bf, bias_row = _host_prep(inputs)
    in_maps = [dict(xt=xts[i], wt=wt_bf, bias=bias_row)
               for i in range(N_CORES)]
    res = None
    last_exc = None
    for attempt in range(3):
        try:
            res = run_bass_kernel_spmd(nc, in_maps,
                                       core_ids=list(range(N_CORES)))
            break
        except Exception as e:  # transient device/runtime hiccups
            last_exc = e
            time.sleep(2.0)
    if res is None:
        raise last_exc
    out = np.concatenate([res.results[i]["out"] for i in range(N_CORES)],
                         axis=0)
    return np.ascontiguousarray(out.astype(np.float32))


# revision 14
# speedup vs baseline: 1.0650x; 1.0009x over previous
"""Trainium2 Bass kernel for nn_ExpandingLinear.

Reference computation:
    x_exp = concat([x, x[:, p0] * v0, x_exp1[:, p1] * v1], axis=1)   # [B, 2176]
    W     = scatter_add(weight_vals at [weight_rows, weight_cols])    # [2048, 2176]
    b     = scatter_add(bias_vals at bias_idx)                        # [2048]
    out   = x_exp @ W.T + b                                           # [B, 2048]

Every expanded feature c is a_c * x[:, q_c] for a resolvable (q_c, a_c)
(parent chains only reference earlier features), so the embed columns fold
into the base weight on the host:
    W_eff[o, q_c] += a_c * W[o, 2048 + c]      ->  out = x @ W_eff.T + b
which reduces the device work to a dense [1024, 2048] @ [2048, 2048]
matmul + bias per core (data-parallel batch shard, 8 cores).

Numerics: x and W_eff are cast to bf16 on the host (PSUM accumulates fp32);
measured end-to-end rel err ~4e-3 against the fp32 reference, well inside
the 2e-2 gate, and bf16 halves every DMA stream vs fp32.

Device schedule (per core):
  - wt (W_eff^T, [16 k-tiles, 128, 2048]) and xt (x^T, [16, 128, 1024])
    stream into resident SBUF tiles as 256KB k-tiles on 4 queues:
    W n-cols 0:1024 on sync/scalar (k even/odd), x on gpsimd/vector.
    W n-cols 1024:2048 and the bias queue behind those (needed later).
  - round n=0 runs k-outer / m-inner, paced by the k-ordered W/x streams;
    all 8 PSUM banks accumulate one m-tile each.
  - rounds n=1..3 run m-outer / k-inner from resident SBUF, staggering
    PSUM completion so evac (vector add bias -> bf16) + store overlap
    the next m-block's matmuls.
"""

import numpy as np
from contextlib import ExitStack

OUT = 2048
IN_BASE = 2048
N_EMBED = 64
IN_TOT = IN_BASE + 2 * N_EMBED  # 2176
BATCH = 8192
N_CORES = 8
B_CORE = BATCH // N_CORES       # 1024
P = 128
K_TILES = IN_BASE // P          # 16 (embeds folded away)
M_TILES = B_CORE // P           # 8
N_SPLIT = 4                     # 2048 out cols in 4 x 512 (one PSUM bank each)
NW = 512

_CACHED = {}


def _build_nc():
    import concourse.mybir as mybir
    import concourse.tile as tile
    from concourse import bacc

    f32 = mybir.dt.float32
    bf16 = mybir.dt.bfloat16

    nc = bacc.Bacc("TRN2", target_bir_lowering=False, debug=False,
                   num_devices=N_CORES)

    xt = nc.dram_tensor("xt", [IN_BASE, B_CORE], bf16, kind="ExternalInput")
    wt = nc.dram_tensor("wt", [IN_BASE, OUT], bf16, kind="ExternalInput")
    bias = nc.dram_tensor("bias", [1, OUT], f32, kind="ExternalInput")
    out = nc.dram_tensor("out", [B_CORE, OUT], bf16, kind="ExternalOutput")

    xt_ap = xt.ap().rearrange("(k p) b -> p k b", p=P)   # [128, 16, 1024]
    wt_ap = wt.ap().rearrange("(k p) n -> p k n", p=P)   # [128, 16, 2048]

    NHALF = OUT // 2  # 1024

    with tile.TileContext(nc) as tc:
        with ExitStack() as ctx:
            big_pool = ctx.enter_context(tc.tile_pool(name="big", bufs=1))
            out_pool = ctx.enter_context(tc.tile_pool(name="out", bufs=4))
            psum_pool = ctx.enter_context(
                tc.tile_pool(name="psum", bufs=8, space="PSUM"))

            wt_sb = big_pool.tile([P, K_TILES * OUT], bf16, tag="wt")
            xt_sb = big_pool.tile([P, K_TILES * B_CORE], bf16, tag="xt")
            bias_row = big_pool.tile([1, OUT], f32, tag="bias_row")
            bias_t = big_pool.tile([P, OUT], f32, tag="bias")

            # PE p-state warmup: the Tensor engine ramps 0.65 -> 1.2 ->
            # 2.4 GHz over ~3us of continuous execution. Small junk
            # matmuls issued before the input streams land burn the ramp
            # so the real matmuls start at full clock. 256-row moving dim
            # keeps the preemption granularity ~0.1-0.2us.
            warm_pool = ctx.enter_context(tc.tile_pool(name="warm", bufs=1))
            wx = warm_pool.tile([P, P], bf16, tag="wx")
            ww = warm_pool.tile([P, 256], bf16, tag="ww")
            nc.vector.memset(wx[:], 0.0)
            nc.vector.memset(ww[:], 0.0)
            wps = psum_pool.tile([P, NW], f32, tag="ps", name="warm_ps")
            for _ in range(20):
                nc.tensor.matmul(wps[:, 0:256], lhsT=wx[:], rhs=ww[:],
                                 start=True, stop=True)

            # k-ordered streams paced for round 0: x k-tiles (256KB per
            # ~1.7us k-step) on gpsimd, the W n0 quarter (128KB/step) on
            # scalar; n1 and the n-half-B chunks queue behind on sync/
            # scalar for rounds 1-3. dma_start CREATION order tracks
            # consumption order: the runtime round-robins a small pool of
            # DMA-completion semaphores over dma_starts in program order,
            # so a transfer can only begin once the 8-ago transfer is
            # done — out-of-need-order issues serialize the critical path
            # behind bulk prefetch.
            nc.sync.dma_start(
                out=xt_sb[:, 0:B_CORE // 2], in_=xt_ap[:, 0, 0:B_CORE // 2])
            nc.scalar.dma_start(
                out=wt_sb[:, 0:NW], in_=wt_ap[:, 0, 0:NW])
            nc.sync.dma_start(
                out=xt_sb[:, B_CORE // 2:B_CORE],
                in_=xt_ap[:, 0, B_CORE // 2:B_CORE])
            nc.sync.dma_start(out=bias_row[:], in_=bias.ap())
            for k in range(1, K_TILES):
                nc.gpsimd.dma_start(
                    out=xt_sb[:, k * B_CORE:(k + 1) * B_CORE],
                    in_=xt_ap[:, k, :])
                nc.scalar.dma_start(
                    out=wt_sb[:, k * OUT:k * OUT + NW],
                    in_=wt_ap[:, k, 0:NW])
            for k in range(K_TILES):
                nc.sync.dma_start(
                    out=wt_sb[:, k * OUT + NW:k * OUT + NHALF],
                    in_=wt_ap[:, k, NW:NHALF])
            for k in range(K_TILES):
                eng = nc.scalar if k % 2 == 0 else nc.sync
                eng.dma_start(
                    out=wt_sb[:, k * OUT + NHALF:(k + 1) * OUT],
                    in_=wt_ap[:, k, NHALF:OUT])
            nc.gpsimd.partition_broadcast(bias_t[:], bias_row[:])

            def lhsT(k, m):
                return xt_sb[:, k * B_CORE + m * P:k * B_CORE + (m + 1) * P]

            def rhs(k, n):
                return wt_sb[:, k * OUT + n * NW:k * OUT + (n + 1) * NW]

            def evac(psum, m, n):
                ot = out_pool.tile([P, NW], bf16, tag="ot")
                nc.vector.tensor_add(
                    ot[:], psum[:], bias_t[:, n * NW:(n + 1) * NW])
                # late-round stores ride the HWDGE queues, idle once the
                # W stream ends — the SWDGE (gpsimd) end-of-kernel drain
                # is ~5us when stores are still in its queue
                if n >= 2:
                    st = nc.sync if m % 2 == 0 else nc.scalar
                else:
                    st = nc.gpsimd
                st.dma_start(
                    out=out.ap()[m * P:(m + 1) * P, n * NW:(n + 1) * NW],
                    in_=ot[:])

            # round n=0: k-outer so PE consumes W/x k-tiles in arrival
            # order (~240 GB/s joint stream pace, under the ~350 measured)
            psums = [psum_pool.tile([P, NW], f32, tag="ps",
                                    name=f"ps_n0_m{m}")
                     for m in range(M_TILES)]
            for k in range(K_TILES):
                for m in range(M_TILES):
                    nc.tensor.matmul(
                        psums[m][:], lhsT=lhsT(k, m), rhs=rhs(k, 0),
                        start=(k == 0), stop=(k == K_TILES - 1))
            for m in range(M_TILES):
                evac(psums[m], m, 0)

            # rounds n=1..3: resident SBUF, m-outer staggers psum
            # completion so evac+store overlap the next m-block
            for n in range(1, N_SPLIT):
                for m in range(M_TILES):
                    ps = psum_pool.tile([P, NW], f32, tag="ps",
                                        name=f"ps_n{n}_m{m}")
                    last = (n == N_SPLIT - 1 and m == M_TILES - 1)
                    if last:
                        # split the final 512-col group into two 256-col
                        # accumulation halves so the first half's
                        # evac+store overlaps the second half's matmuls,
                        # shortening the serial drain after the last mm
                        for h in range(2):
                            hs = slice(h * (NW // 2), (h + 1) * (NW // 2))
                            for k in range(K_TILES):
                                nc.tensor.matmul(
                                    ps[:, hs],
                                    lhsT=lhsT(k, m),
                                    rhs=rhs(k, n)[:, hs],
                                    start=(k == 0),
                                    stop=(k == K_TILES - 1))
                            ot = out_pool.tile([P, NW // 2], bf16, tag="oth")
                            nc.vector.tensor_add(
                                ot[:], ps[:, hs],
                                bias_t[:, n * NW + h * (NW // 2):
                                       n * NW + (h + 1) * (NW // 2)])
                            st = nc.sync if h == 0 else nc.scalar
                            st.dma_start(
                                out=out.ap()[m * P:(m + 1) * P,
                                             n * NW + h * (NW // 2):
                                             n * NW + (h + 1) * (NW // 2)],
                                in_=ot[:])
                    else:
                        for k in range(K_TILES):
                            nc.tensor.matmul(
                                ps[:], lhsT=lhsT(k, m), rhs=rhs(k, n),
                                start=(k == 0), stop=(k == K_TILES - 1))
                        evac(ps, m, n)

    nc.compile()
    return nc


def _host_prep(inputs):
    import ml_dtypes

    x = np.asarray(inputs["x"], dtype=np.float32)
    wv = np.asarray(inputs["weight_vals"], dtype=np.float32)
    wr = np.asarray(inputs["weight_rows"]).astype(np.int64)
    wc = np.asarray(inputs["weight_cols"]).astype(np.int64)
    bv = np.asarray(inputs["bias_vals"], dtype=np.float32)
    bi = np.asarray(inputs["bias_idx"]).astype(np.int64)
    e0v = np.asarray(inputs["embed0_vals"], dtype=np.float32)
    e0p = np.asarray(inputs["embed0_parents"]).astype(np.int64)
    e1v = np.asarray(inputs["embed1_vals"], dtype=np.float32)
    e1p = np.asarray(inputs["embed1_parents"]).astype(np.int64)

    # dense W^T [IN_TOT, OUT] (coalesce: duplicates sum)
    wt_full = np.bincount(wc * OUT + wr, weights=wv,
                          minlength=IN_TOT * OUT).reshape(IN_TOT, OUT)

    # resolve embed parent chains to (row-in-x, multiplier), then fold the
    # expanded-feature rows of W^T into their parent rows
    q = np.empty(2 * N_EMBED, dtype=np.int64)
    a = np.empty(2 * N_EMBED, dtype=np.float64)
    q[:N_EMBED] = e0p
    a[:N_EMBED] = e0v
    for j in range(N_EMBED):
        p = int(e1p[j])
        if p < IN_BASE:
            q[N_EMBED + j] = p
            a[N_EMBED + j] = e1v[j]
        else:
            t = p - IN_BASE
            q[N_EMBED + j] = e0p[t]
            a[N_EMBED + j] = e1v[j] * e0v[t]
    wt_eff = wt_full[:IN_BASE]
    np.add.at(wt_eff, q, a[:, None] * wt_full[IN_BASE:])
    wt_bf = np.ascontiguousarray(wt_eff.astype(ml_dtypes.bfloat16))

    b = np.bincount(bi, weights=bv, minlength=OUT).astype(np.float32)
    bias_row = np.ascontiguousarray(b[None, :])

    x_bf = x.astype(ml_dtypes.bfloat16)
    xts = [np.ascontiguousarray(x_bf[i * B_CORE:(i + 1) * B_CORE].T)
           for i in range(N_CORES)]
    return xts, wt_bf, bias_row


def kernel(**inputs) -> np.ndarray:
    import time
    from concourse.bass_utils import run_bass_kernel_spmd

    if "nc" not in _CACHED:
        _CACHED["nc"] = _build_nc()
    nc = _CACHED["nc"]

    xts, wt_bf, bias_row = _host_prep(inputs)
    in_maps = [dict(xt=xts[i], wt=wt_bf, bias=bias_row)
               for i in range(N_CORES)]
    res = None
    last_exc = None
    for attempt in range(3):
        try:
            res = run_bass_kernel_spmd(nc, in_maps,
                                       core_ids=list(range(N_CORES)))
            break
        except Exception as e:  # transient device/runtime hiccups
            last_exc = e
            time.sleep(2.0)
    if res is None:
        raise last_exc
    out = np.concatenate([res.results[i]["out"] for i in range(N_CORES)],
                         axis=0)
    return np.ascontiguousarray(out.astype(np.float32))
